# revision 7
# baseline (speedup 1.0000x reference)
"""CacheFuser Trainium2 Bass kernel (v3).

Sharding: layer-parallel — 8 layers -> 8 NeuronCores, one layer per core.

Design (see git history for v1/v2):
  * All tensors pre-transposed to feature-major tiled layout on the HOST, so
    the kernel does ZERO on-chip transposes.
  * fp8 e4m3 (host-cast) for: sharer data + w1 (aligner matmuls, DoubleRow),
    aggregate G + w2p (fusion second half, DoubleRow), F + fw2 (final matmul,
    DoubleRow). Receiver/residual/output stay fp16.  Measured end-to-end
    rel-err ~1.1e-2 vs the 2e-2 gate (numpy sim matches HW to 4 digits).
  * ReLU identity max(x+b,0) = max(x,-b)+b turns per-sharer bias+ReLU+
    aggregate into single DVE scalar_tensor_tensor chain steps, with bias
    sums folded into downstream biases on the host.
  * out = gate*(F@fw2) + r_pre in one DVE stt from PSUM (fb2 folded into
    r_pre on the host). Output written fp16, upcast on host.
  * Software pipelining: fusion+final of tile it-1 are interleaved between
    the aligner groups of tile it, so the strict-FIFO PE queue always has
    ready work while aligner PSUM buffers wait on ACT/DVE consumers.

Engine split per tile (TS=512 tokens):  PE 32 matmuls; ACT 4 aligner ReLUs +
2 fusion ReLUs; DVE 2 merges + 4 chain steps + 2 residual stt; GpSimd the
output store.
"""
import sys

sys.path.insert(0, "/opt/trn_rl_repo")

import numpy as np
import ml_dtypes

L, N, B, S, H = 8, 4, 2, 4096, 256
T = B * S
TAU = 0.5
TS = 512           # tokens per tile iteration
NT = T // TS       # 16 iterations

# per cache: which sharers go through ACT true-bias ReLU (+ merges) vs the
# DVE max-shift chain.  c0 leans ACT-heavy, c1 DVE-heavy, to balance engines.
ACT_NS_C = {0: (0, 1, 2), 1: (0, 1)}
CHAIN_NS_C = {0: (3,), 1: (2, 3)}

_CACHE = {}


def _build_program(zb: bool):
    """zb=True: folded bias vectors are all zero -> immediate-0 fast path with
    full [128, 2, TS] elementwise instructions. zb=False: general path with
    per-m [128, TS] instructions and per-partition bias APs."""
    import concourse.bacc as bacc
    import concourse.mybir as mybir
    from concourse.tile import TileContext

    F32 = mybir.dt.float32
    F16 = mybir.dt.float16
    F8 = mybir.dt.float8e4
    Relu = mybir.ActivationFunctionType.Relu
    MAX = mybir.AluOpType.max
    ADD = mybir.AluOpType.add
    MULT = mybir.AluOpType.mult
    DR = mybir.MatmulPerfMode.DoubleRow

    nc = bacc.Bacc()

    sx_d = nc.declare_dram_parameter("sx", [NT, 128, 2, N, 2, TS], F8, isOutput=False)
    rx_d = nc.declare_dram_parameter("rx", [NT, 128, 2, 2, TS], F16, isOutput=False)
    rx8_d = nc.declare_dram_parameter("rx8", [NT, 128, 2, 2, TS], F8, isOutput=False)
    out_d = nc.declare_dram_parameter("out", [NT, 128, 2, 2, TS], F16, isOutput=True)
    w18_d = [nc.declare_dram_parameter(f"w18{c}", [128, 2, H], F8, isOutput=False)
             for c in (0, 1)]
    w2p8_d = [nc.declare_dram_parameter(f"w2p8{c}", [128, 2, H], F8, isOutput=False)
              for c in (0, 1)]
    fw28_d = [nc.declare_dram_parameter(f"fw28{c}", [128, 2, H], F8, isOutput=False)
              for c in (0, 1)]
    fw1a8_d = [nc.declare_dram_parameter(f"fw1a8{c}", [128, 2, H], F8, isOutput=False)
               for c in (0, 1)]
    gate_d = nc.declare_dram_parameter("gate", [128, 1], F32, isOutput=False)
    if not zb:
        ab1_d = nc.declare_dram_parameter("ab1", [128, 2, N, 2], F32, isOutput=False)
        nb1_d = nc.declare_dram_parameter("nb1", [128, 2, N, 2], F32, isOutput=False)
        fb1e_d = nc.declare_dram_parameter("fb1e", [128, 2, 2], F32, isOutput=False)

    with TileContext(nc) as tc:
        with tc.tile_pool(name="const", bufs=1) as cpool, \
             tc.tile_pool(name="io", bufs=3) as iop, \
             tc.tile_pool(name="act", bufs=2) as apool, \
             tc.tile_pool(name="psA", bufs=2, space="PSUM") as psA, \
             tc.tile_pool(name="psFD", bufs=2, space="PSUM") as psFD:

            def cload(d, dt, tag):
                t_ = cpool.tile([128, 2, H], dt, tag=tag)
                nc.sync.dma_start(out=t_, in_=d[...])
                return t_

            w18 = [cload(w18_d[c], F8, f"w18{c}") for c in (0, 1)]
            w2p8 = [cload(w2p8_d[c], F8, f"w2p8{c}") for c in (0, 1)]
            fw28 = [cload(fw28_d[c], F8, f"fw28{c}") for c in (0, 1)]
            fw1a8 = [cload(fw1a8_d[c], F8, f"fw1a8{c}") for c in (0, 1)]
            gate_t = cpool.tile([128, 1], F32)
            nc.sync.dma_start(out=gate_t, in_=gate_d[...])
            if not zb:
                ab1_t = cpool.tile([128, 2, N, 2], F32, tag="ab1")
                nc.sync.dma_start(out=ab1_t, in_=ab1_d[...])
                nb1_t = cpool.tile([128, 2, N, 2], F32, tag="nb1")
                nc.sync.dma_start(out=nb1_t, in_=nb1_d[...])
                fb1e_t = cpool.tile([128, 2, 2], F32, tag="fb1e")
                nc.sync.dma_start(out=fb1e_t, in_=fb1e_d[...])

            def act_relu(dst, ps, bias_ap):
                if zb:
                    nc.scalar.activation(out=dst, in_=ps, func=Relu)
                else:
                    for m in range(2):
                        nc.scalar.activation(out=dst[:, m, :], in_=ps[:, m, :],
                                             func=Relu, bias=bias_ap(m))

            def chain_step(dst, ps, src, nscal_ap):
                """dst = max(ps, -b) + src on DVE."""
                if zb:
                    nc.vector.scalar_tensor_tensor(
                        out=dst, in0=ps, scalar=0.0, in1=src, op0=MAX, op1=ADD)
                else:
                    for m in range(2):
                        nc.vector.scalar_tensor_tensor(
                            out=dst[:, m, :], in0=ps[:, m, :], scalar=nscal_ap(m),
                            in1=src[:, m, :], op0=MAX, op1=ADD)

            def aligner_group(sx, n, c, hn, G, Gm, G8):
                """2 DR matmuls + consumer for sharer n of cache c.
                c0: ACT n0,n1,n2; Gm = hn0+hn1 (GPS, off critical path);
                    G = Gm+hn2 (DVE); chain n3 -> G8.
                c1: ACT n0,n1; G = hn0+hn1 (DVE); chains n2 -> G, n3 -> G8."""
                ps = psA.tile([128, 2, TS], F32, tag="al")
                for m in range(2):
                    nc.tensor.matmul(ps[:, m, :],
                                     lhsT=w18[c][:, :, m * 128:(m + 1) * 128],
                                     rhs=sx[:, c, n, :, :],
                                     start=True, stop=True, perf_mode=DR)
                a_ns, ch_ns = ACT_NS_C[c], CHAIN_NS_C[c]
                if n in a_ns:
                    act_relu(hn[n], ps, lambda m: ab1_t[:, c, n, m:m + 1])
                    if c == 0 and n == 1:
                        nc.gpsimd.tensor_tensor(out=Gm, in0=hn[0], in1=hn[1],
                                                op=ADD)
                    elif c == 0 and n == 2:
                        nc.vector.tensor_tensor(out=G, in0=Gm, in1=hn[2], op=ADD)
                    elif c == 1 and n == 1:
                        nc.vector.tensor_tensor(out=G, in0=hn[0], in1=hn[1],
                                                op=ADD)
                elif n == ch_ns[-1]:
                    chain_step(G8, ps, G, lambda m: nb1_t[:, c, n, m:m + 1])
                else:
                    chain_step(G, ps, G, lambda m: nb1_t[:, c, n, m:m + 1])

            def fusion_half(pp, rx8, G8, c, m):
                """2 DR matmuls: P[:, m] = rx8_c @ fw1a8 + G8 @ w2p8."""
                sl = slice(m * 128, (m + 1) * 128)
                nc.tensor.matmul(pp[:, m, :], lhsT=fw1a8[c][:, :, sl],
                                 rhs=rx8[:, c], start=True, stop=False,
                                 perf_mode=DR)
                nc.tensor.matmul(pp[:, m, :], lhsT=w2p8[c][:, :, sl],
                                 rhs=G8, start=False, stop=True, perf_mode=DR)

            def final_piece(pp, F8_t, rx, o16, c):
                """F8 @ fw2 (DR, reusing pp) then out = gate*pd + r_pre."""
                for m in range(2):
                    nc.tensor.matmul(pp[:, m, :],
                                     lhsT=fw28[c][:, :, m * 128:(m + 1) * 128],
                                     rhs=F8_t, start=True, stop=True, perf_mode=DR)
                nc.vector.scalar_tensor_tensor(
                    out=o16[:, c], in0=pp, scalar=gate_t[:, 0:1],
                    in1=rx[:, c], op0=MULT, op1=ADD)

            st = {}
            for it in range(NT + 1):
                if it < NT:
                    sx = iop.tile([128, 2, N, 2, TS], F8, tag="sx", bufs=4)
                    nc.sync.dma_start(out=sx, in_=sx_d[it])
                    rx = iop.tile([128, 2, 2, TS], F16, tag="rx", bufs=4)
                    nc.sync.dma_start(out=rx, in_=rx_d[it])
                    rx8 = iop.tile([128, 2, 2, TS], F8, tag="rx8", bufs=4)
                    nc.sync.dma_start(out=rx8, in_=rx8_d[it])
                    hn = {c: {n: apool.tile([128, 2, TS], F16, tag=f"hn{n}{c}",
                                            name=f"hn{n}{c}")
                              for n in ACT_NS_C[c]} for c in (0, 1)}
                    G = {c: apool.tile([128, 2, TS], F16, tag=f"G{c}",
                                       name=f"G{c}")
                         for c in (0, 1)}
                    Gm = apool.tile([128, 2, TS], F16, tag="Gm", name="Gm")
                    G8 = {c: apool.tile([128, 2, TS], F8, tag=f"G8{c}",
                                        name=f"G8{c}")
                          for c in (0, 1)}
                    cur = {"sx": sx, "rx": rx, "rx8": rx8, "hn": hn,
                           "G": G, "Gm": Gm, "G8": G8}
                else:
                    cur = None

                prv = st.pop(it - 1, None)
                if prv is not None:
                    prv["pp"] = {}
                    prv["F8"] = {}
                    prv["o16"] = iop.tile([128, 2, 2, TS], F16, tag="o16", bufs=2, name="o16")

                # interleave: aligner groups of tile `it` with fusion/final
                # pieces of tile `it-1` (pieces are ready work that absorbs
                # PE stalls on aligner PSUM rotation)
                def piece(i):
                    if prv is None:
                        return
                    pG8, prx, po = prv["G8"], prv["rx"], prv["o16"]
                    prx8 = prv["rx8"]
                    if i in (0, 2):          # fusion m=0 of cache k / v
                        c = 0 if i == 0 else 1
                        pp = psFD.tile([128, 2, TS], F32, tag="fd", name="pp")
                        prv["pp"][c] = pp
                        fusion_half(pp, prx8, pG8[c], c, 0)
                    elif i in (1, 3):        # fusion m=1 + F ReLU
                        c = 0 if i == 1 else 1
                        pp = prv["pp"][c]
                        fusion_half(pp, prx8, pG8[c], c, 1)
                        F8_t = apool.tile([128, 2, TS], F8, tag=f"F8{c}", name=f"F8{c}")
                        prv["F8"][c] = F8_t
                        act_relu(F8_t, pp, lambda m, c=c: fb1e_t[:, c, m:m + 1])
                    elif i in (4, 5):        # final + residual + (store)
                        c = 0 if i == 4 else 1
                        final_piece(prv["pp"][c], prv["F8"][c], prx, po, c)
                        if i == 5:
                            nc.gpsimd.dma_start(out=out_d[it - 1], in_=po)

                if cur is not None:
                    order = [(0, 0), (0, 1), "p0", (1, 0), "p1", (1, 1), "p2",
                             (2, 0), "p3", (2, 1), "p4", (3, 0), "p5", (3, 1)]
                    for step in order:
                        if isinstance(step, str):
                            piece(int(step[1]))
                        else:
                            n, c = step
                            aligner_group(cur["sx"], n, c, cur["hn"][c],
                                          cur["G"][c], cur["Gm"], cur["G8"][c])
                    st[it] = cur
                else:
                    for i in range(6):
                        piece(i)

    nc.finalize()
    return nc


def _sigmoid(x):
    return 1.0 / (1.0 + np.exp(-x))


def _pm(vec):
    """[H] vector -> [128, 2] partition-major (h = m*128 + p)."""
    return np.ascontiguousarray(np.asarray(vec, np.float32).reshape(2, 128).T)


def _wt(mat, dt):
    """[H, H] weight -> [128, 2, H] lhsT tiles (contraction chunk on part)."""
    return np.ascontiguousarray(
        np.asarray(mat, np.float32).reshape(2, 128, H).transpose(1, 0, 2)).astype(dt)


def _feat_major(x):
    """[T, H] -> [NT, 128, 2, TS]  (tile, p, kc, ts) with h = kc*128 + p."""
    return x.reshape(NT, TS, 2, 128).transpose(0, 3, 2, 1)


def _prep_layer(inputs, l):
    f16 = np.float16
    f8 = ml_dtypes.float8_e4m3fn
    e = np.asarray(inputs["edge_weights"][l], np.float32)
    esc = e / N
    g = float(_sigmoid(float(inputs["alpha"][l]) / TAU))
    m = {"gate": np.full((128, 1), g, np.float32)}

    sx_c, rx_c = [], []
    ab1 = np.zeros((128, 2, N, 2), np.float32)
    nb1 = np.zeros((128, 2, N, 2), np.float32)
    fb1e = np.zeros((128, 2, 2), np.float32)
    for c, (rk, sk, p) in enumerate([("receiver_k", "sharer_k", "ak"),
                                     ("receiver_v", "sharer_v", "av")]):
        fp = "fk" if c == 0 else "fv"
        R = np.asarray(inputs[rk][l], np.float32).reshape(T, H)
        X = np.asarray(inputs[sk][l], np.float32).reshape(N, T, H)
        w1 = np.asarray(inputs[f"{p}_w1"][l], np.float32)
        b1 = np.asarray(inputs[f"{p}_b1"][l], np.float32)
        w2 = np.asarray(inputs[f"{p}_w2"][l], np.float32)
        b2 = np.asarray(inputs[f"{p}_b2"][l], np.float32)
        fw1 = np.asarray(inputs[f"{fp}_w1"][l], np.float32)
        fb1 = np.asarray(inputs[f"{fp}_b1"][l], np.float32)
        fw2 = np.asarray(inputs[f"{fp}_w2"][l], np.float32)
        fb2 = np.asarray(inputs[f"{fp}_b2"][l], np.float32)
        fw1a, fw1b = fw1[:H], fw1[H:]
        w2p = w2 @ fw1b

        # bias folds (see module docstring)
        cshift = sum(esc[n] for n in CHAIN_NS_C[c]) * b1     # chain shift
        pbias = fb1 + esc.sum() * (b2 @ fw1b) + cshift @ w2p
        pbias_adj = pbias - g * (fb2 @ fw1a)
        r_pre = R + g * fb2[None, :]

        for n in ACT_NS_C[c]:
            ab1[:, c, n, :] = _pm(esc[n] * b1)
        for n in CHAIN_NS_C[c]:
            nb1[:, c, n, :] = _pm(-esc[n] * b1)
        fb1e[:, c, :] = _pm(pbias_adj)

        Xs = X * esc[:, None, None]
        sx_c.append(Xs.reshape(N, NT, TS, 2, 128).transpose(1, 4, 0, 3, 2))
        rx_c.append(_feat_major(r_pre))

        m[f"w18{c}"] = _wt(w1, f8)
        m[f"w2p8{c}"] = _wt(w2p, f8)
        m[f"fw1a8{c}"] = _wt(fw1a, f8)
        m[f"fw28{c}"] = _wt(fw2, f8)

    m["sx"] = np.ascontiguousarray(np.stack(sx_c, axis=2)).astype(f8)
    rxs = np.ascontiguousarray(np.stack(rx_c, axis=2))
    m["rx"] = rxs.astype(f16)
    m["rx8"] = rxs.astype(f8)
    m["ab1"], m["nb1"], m["fb1e"] = ab1, nb1, fb1e
    return m


def _prep_in_maps(inputs):
    from concurrent.futures import ThreadPoolExecutor
    with ThreadPoolExecutor(max_workers=8) as ex:
        in_maps = list(ex.map(lambda l: _prep_layer(inputs, l), range(L)))
    zb = all(
        float(np.abs(m[k]).max()) == 0.0
        for m in in_maps for k in ("ab1", "nb1", "fb1e"))
    if zb:
        for m in in_maps:
            del m["ab1"], m["nb1"], m["fb1e"]
    return in_maps, zb


def _unpack_out(res_l):
    """[NT, 128, 2, 2, TS] f16 -> [2, T, H] f32."""
    r = np.asarray(res_l).astype(np.float32)
    return r.transpose(2, 0, 4, 3, 1).reshape(2, T, H)


def _run(inputs, trace=False):
    from concourse.bass_utils import run_bass_kernel_spmd

    in_maps, zb = _prep_in_maps(inputs)
    key = f"nc{zb}"
    if key not in _CACHE:
        _CACHE[key] = _build_program(zb)
    nc = _CACHE[key]
    res = run_bass_kernel_spmd(nc, in_maps, list(range(L)), trace=trace)
    from concurrent.futures import ThreadPoolExecutor
    with ThreadPoolExecutor(max_workers=8) as ex:
        outs = list(ex.map(lambda l: _unpack_out(res.results[l]["out"]), range(L)))
    full = np.stack(outs, axis=1)                                # [2, L, T, H]
    return full.reshape(2, L, B, S, H).astype(np.float32), res


def kernel(**inputs):
    out, _ = _run(inputs, trace=False)
    return out


def kernel_traced(**inputs):
    """Like kernel() but also returns the profiled hardware exec time (ns)."""
    out, res = _run(inputs, trace=True)
    return out, res.exec_time_ns
